# revision 1
# baseline (speedup 1.0000x reference)
# Trainium2 Bass kernel for NormalAttention (1x1-conv q/k/v attention over HW).
#
# Math (per batch b, N = 4096 spatial positions):
#   q = Wq x + bq            [64, N]
#   k = Wk x + bk            [64, N]
#   E[i,j] = sum_c q[c,i] k[c,j]
#   A = elu(E) / N
#   y = Wg ((Wv x + bv) @ A) + bg
#
# Kernel strategy: data-parallel, one batch per NeuronCore (8 cores).
#
# Key restructurings vs the reference math:
#  1. Gamma conv fused into the value conv on the host:
#       y = ((Wg Wv) x + Wg bv) @ A + bg
#     Device computes one fused conv u = Wgv x + bvg; the attention matmul
#     accumulates y directly in PSUM. ut = u^T stored transposed [N, C] as
#     the stationary operand (fp8, unscaled; the 1/N rides the epilogue copy).
#  2. elu in TWO elementwise passes via the exact identity
#       elu(E) + 1 = max(min(exp(E), 1), E + 1)
#     E+1 comes free out of the energy matmul: q,k get a 65th ones channel
#     (K=65), so PSUM holds E' = E+1. Then
#       t = exp(E' - 1)           (ACT, -1 via the activation bias)
#       g = (t min 1.0) max E'    (DVE scalar_tensor_tensor)
#  3. The +1 offset correction bge = bg - rowsum(ut)/N is host-precomputed
#     (needs only x.sum(spatial)) and folded into the y accumulation as a
#     K=1 rank-1 matmul, so the epilogue is one ACT copy + one DMA per mt.
#  4. Out-matmul runs fp8e4m3 with perf_mode=DoubleRow: one matmul contracts
#     both 128-row n-chunks (K=256) via [128, 2, .] interleaved APs.
import os

import numpy as np
import ml_dtypes

import concourse.bass as bass
import concourse.mybir as mybir
import concourse.tile as tile
from concourse import bacc
from concourse.bass_utils import run_bass_kernel_spmd

B, C, HH, WW = 8, 256, 64, 64
N = HH * WW          # 4096 spatial positions
CQ = 64              # query/key channels
CQ1 = CQ + 1         # +1 ones channel (makes the matmul produce E+1)
NCORES = 8
MT = 512             # m (energy column) tile
NPAIRS = 16          # pairs of 128-row n-chunks per m-tile
VARIANT = os.environ.get("KVAR", "full")  # full | noelem | noout | noexp
FP8 = os.environ.get("KFP8", "1") == "1"  # fp8 DoubleRow out-matmul

F32 = mybir.dt.float32
F32R = mybir.dt.float32r
BF16 = mybir.dt.bfloat16
F8 = mybir.dt.float8e4
AL = mybir.AluOpType
AF = mybir.ActivationFunctionType
PM = mybir.MatmulPerfMode


def build_nc(reps=1, variant=None):
    global VARIANT
    if variant is not None:
        VARIANT = variant
    nc = bacc.Bacc("TRN2", target_bir_lowering=False, debug=False,
                   num_devices=NCORES)
    xd = nc.declare_dram_parameter("x", [2, 128, N], F32R, isOutput=False)
    wqd = nc.declare_dram_parameter("wqT", [2, 128, CQ], F32R, isOutput=False)
    wkd = nc.declare_dram_parameter("wkT", [2, 128, CQ], F32R, isOutput=False)
    bqd = nc.declare_dram_parameter("bq", [CQ, 1], F32, isOutput=False)
    bkd = nc.declare_dram_parameter("bk", [CQ, 1], F32, isOutput=False)
    wgvd = nc.declare_dram_parameter("wgvT", [2, 128, C], F32R, isOutput=False)
    bvgd = nc.declare_dram_parameter("bvg", [1, C], F32R, isOutput=False)
    bged = nc.declare_dram_parameter("bgeN", [1, C], BF16, isOutput=False)
    onesd = nc.declare_dram_parameter("onesN", [1, N], BF16, isOutput=False)
    ones128 = nc.declare_dram_parameter("ones128", [1, 128], F32R,
                                        isOutput=False)
    od = nc.declare_dram_parameter("out", [128, 2, N], F32, isOutput=True)

    with tile.TileContext(nc) as tc:
        with (
            tc.tile_pool(name="wts", bufs=1) as wts,
            # xs/qk/ut double-buffered so iteration i+1's input DMA + convs
            # overlap iteration i's tail in the timing For_i loop (their
            # single-buffered WAR deps otherwise serialize on i's LAST reads)
            tc.tile_pool(name="xs", bufs=1) as xs,
            tc.tile_pool(name="qk", bufs=1) as qkp,
            tc.tile_pool(name="ut", bufs=1) as utp,
            tc.tile_pool(name="tp", bufs=4) as tpp,
            tc.tile_pool(name="gp", bufs=6) as gpp,
            tc.tile_pool(name="fo", bufs=2) as fop,
            tc.tile_pool(name="pse", bufs=3, space="PSUM") as pse,
            tc.tile_pool(name="psy", bufs=1, space="PSUM") as psy,
        ):
            def body(iv=None):
                # small weight/bias DMAs go through the Pool queue (cheap
                # sequencer dispatch) so the SP queue starts x immediately
                x_sb = [xs.tile([128, N], F32R, tag=f"x{i}", name=f"x_sb{i}")
                        for i in range(2)]
                for i in range(2):   # x batch 0 first: biggest early transfer
                    nc.sync.dma_start(x_sb[i][:, 0:1024], xd[i][:, 0:1024])
                # q/k with a 65th all-ones channel (row 64); everything the
                # first energy matmul needs goes on the SP queue in need-order
                q_sb = qkp.tile([CQ1, N], BF16, tag="q", name="q_sb")
                k_sb = qkp.tile([CQ1, N], BF16, tag="k", name="k_sb")
                nc.sync.dma_start(q_sb[CQ:CQ1, :], onesd[:])
                nc.sync.dma_start(k_sb[CQ:CQ1, :], onesd[:])
                wq_sb = wts.tile([128, 2, CQ], F32R, tag="wq", name="wq_sb")
                wk_sb = wts.tile([128, 2, CQ], F32R, tag="wk", name="wk_sb")
                wgv_sb = wts.tile([128, 2, C], F32R, tag="wgv", name="wgv_sb")
                for i in range(2):
                    nc.sync.dma_start(wq_sb[:, i, :], wqd[i])
                    nc.sync.dma_start(wk_sb[:, i, :], wkd[i])
                bq_sb = wts.tile([CQ, 1], F32, tag="bq", name="bq_sb")
                nc.sync.dma_start(bq_sb, bqd[:])
                bk_sb = wts.tile([CQ, 1], F32, tag="bk", name="bk_sb")
                nc.sync.dma_start(bk_sb, bkd[:])
                # later-needed weights go via the Pool queue
                for i in range(2):
                    nc.gpsimd.dma_start(wgv_sb[:, i, :], wgvd[i])
                bvg_sb = wts.tile([1, C], F32R, tag="bvg", name="bvg_sb")
                nc.gpsimd.dma_start(bvg_sb, bvgd[:])
                bge_sb = wts.tile([1, C], BF16, tag="bge", name="bge_sb")
                nc.gpsimd.dma_start(bge_sb, bged[:])
                ones_row = wts.tile([1, 128], F32R, tag="ones_row",
                                    name="ones_row")
                nc.gpsimd.dma_start(ones_row, ones128[:])
                ones16 = wts.tile([1, MT], BF16, tag="ones16", name="ones16")
                nc.gpsimd.dma_start(ones16, onesd[:, 0:MT])
                ut_sb = utp.tile([128, 32, C], F8 if FP8 else BF16,
                                 tag="ut", name="ut_sb")
                mone = wts.tile([128, 1], F32, tag="mone", name="mone")
                nc.vector.memset(mone, -1.0)
                # dummy first activation: hoists the ~1.3us ACT table load
                # to t~0 instead of just before the first real exp
                dummy = wts.tile([1, 1], F32, tag="dummy", name="dummy")
                nc.scalar.activation(dummy, mone[0:1, :], AF.Exp)

                def emit_x_batch(bi):
                    # 1024 cols of x (both halves)
                    sl = slice(bi * 1024, (bi + 1) * 1024)
                    for i in range(2):
                        nc.sync.dma_start(x_sb[i][:, sl], xd[i][:, sl])

                # ---- q/k conv, 1024-col batches: 8 MMs + 2 ACT copies ----
                def emit_qk_batch(bi):
                    for w, (w_s, b_s, dst) in enumerate(
                            ((wq_sb, bq_sb, q_sb), (wk_sb, bk_sb, k_sb))):
                        ps = pse.tile([128, 2, MT], F32, tag="eps",
                                      name="qkps")
                        for h in range(2):
                            sl = slice(bi * 1024 + h * 512,
                                       bi * 1024 + (h + 1) * 512)
                            nc.tensor.matmul(ps[:CQ, h, :], w_s[:, 0, :],
                                             x_sb[0][:, sl], start=True,
                                             stop=False)
                            nc.tensor.matmul(ps[:CQ, h, :], w_s[:, 1, :],
                                             x_sb[1][:, sl], start=False,
                                             stop=True)
                        nc.scalar.activation(
                            dst[:CQ, bi * 1024:(bi + 1) * 1024],
                            ps[:CQ, :, :], AF.Identity, bias=b_s, scale=1.0)

                # ---- fused value+gamma conv, 4-chunk batches ----
                # ut = x^T WgvT + bvg  (unscaled in fp8 mode)
                def emit_ut_batch(bi):
                    ps = pse.tile([128, 2, MT], F32, tag="eps", name="utps")
                    for ci in range(4):
                        ni = 4 * bi + ci
                        nsl = slice(ni * 128, (ni + 1) * 128)
                        nc.tensor.matmul(ps[:, ci // 2, (ci % 2) * C:
                                            (ci % 2) * C + C],
                                         x_sb[0][:, nsl], wgv_sb[:, 0, :],
                                         start=True, stop=False)
                        nc.tensor.matmul(ps[:, ci // 2, (ci % 2) * C:
                                            (ci % 2) * C + C],
                                         x_sb[1][:, nsl], wgv_sb[:, 1, :],
                                         start=False, stop=False)
                        nc.tensor.matmul(ps[:, ci // 2, (ci % 2) * C:
                                            (ci % 2) * C + C],
                                         ones_row, bvg_sb,
                                         start=False, stop=True)
                    if bi % 4 != 3:
                        nc.scalar.activation(ut_sb[:, 4 * bi:4 * bi + 4, :],
                                             ps[:, :, :], AF.Copy)
                    else:
                        nc.vector.tensor_copy(ut_sb[:, 4 * bi:4 * bi + 4, :],
                                              ps[:, :, :])

                # ---- main attention loop ----
                pairs = [(mt, p) for mt in range(N // MT)
                         for p in range(NPAIRS)]
                g_q = {}
                y_ps = {}

                def emit_e_elem(j):
                    mt, p = pairs[j]
                    msl = slice(mt * MT, (mt + 1) * MT)
                    nA, nB = 2 * p, 2 * p + 1
                    # eps = E + 1 (ones channel included in K=65 contraction)
                    eps = pse.tile([128, 2, MT], F32, tag="eps", name="eps")
                    nc.tensor.matmul(eps[:, 0, :],
                                     q_sb[:, nA * 128:(nA + 1) * 128],
                                     k_sb[:, msl], start=True, stop=True)
                    nc.tensor.matmul(eps[:, 1, :],
                                     q_sb[:, nB * 128:(nB + 1) * 128],
                                     k_sb[:, msl], start=True, stop=True)
                    g16 = gpp.tile([128, 2, MT], F8 if FP8 else BF16,
                                   tag="g", name="g16")
                    if VARIANT == "noelem":
                        nc.vector.tensor_copy(g16[:, :, :], eps[:, :, :])
                        return g16
                    if VARIANT == "noexp":
                        nc.vector.tensor_scalar(g16[:, :, :], eps[:, :, :],
                                                1.0, None, AL.min)
                        return g16
                    t16 = tpp.tile([128, 2, MT], BF16, tag="t", name="t16")
                    # t = exp(E) = exp(eps - 1)
                    nc.scalar.activation(t16[:, :, :], eps[:, :, :], AF.Exp,
                                         bias=mone, scale=1.0)
                    # g = elu(E)+1 = max(min(exp(E), 1), E+1)
                    nc.vector.scalar_tensor_tensor(g16[:, :, :], t16[:, :, :],
                                                   1.0, eps[:, :, :],
                                                   AL.min, AL.max)
                    return g16

                def emit_out(i):
                    mt, p = pairs[i]
                    nA, nB = 2 * p, 2 * p + 1
                    g16 = g_q.pop(i)
                    if VARIANT == "noout":
                        return
                    if p == 0:
                        y_ps[mt] = psy.tile([128, 2, MT], F32, tag="y",
                                            name="y_ps")
                    for ci in range(2):
                        csl = slice(ci * 128, (ci + 1) * 128)
                        if FP8:
                            # one DoubleRow matmul contracts both n-chunks
                            # (K=256 via the [128, 2, .] interleaved APs)
                            nc.tensor.matmul(y_ps[mt][:, ci, :],
                                             ut_sb[:, nA:nB + 1, csl],
                                             g16[:, :, :],
                                             perf_mode=PM.DoubleRow,
                                             start=(p == 0), stop=False)
                        else:
                            nc.tensor.matmul(y_ps[mt][:, ci, :],
                                             ut_sb[:, nA, csl],
                                             g16[:, 0, :], start=(p == 0),
                                             stop=False)
                            nc.tensor.matmul(y_ps[mt][:, ci, :],
                                             ut_sb[:, nB, csl],
                                             g16[:, 1, :], start=False,
                                             stop=False)

                def emit_fin_mm(mt):
                    if VARIANT == "noout":
                        return
                    # bias via K=1 rank-1 matmuls closing the accumulation
                    for ci in range(2):
                        nc.tensor.matmul(y_ps[mt][:, ci, :],
                                         bge_sb[:, ci * 128:(ci + 1) * 128],
                                         ones16[:, :],
                                         start=False, stop=True)

                def emit_fin_copy(mt):
                    if VARIANT == "noout":
                        return
                    # fp8 path: ut was kept unscaled, apply 1/N here
                    sc = (1.0 / N) if FP8 else 1.0
                    msl = slice(mt * MT, (mt + 1) * MT)
                    fo = fop.tile([128, 2, MT], F32, tag="fo", name="fo")
                    nc.scalar.activation(fo[:, :, :], y_ps[mt][:, :, :],
                                         AF.Identity, scale=sc)
                    nc.sync.dma_start(od[:, :, msl], fo[:, :, :])
                    del y_ps[mt]

                emit_qk_batch(0)
                emit_x_batch(1)
                emit_qk_batch(1)
                for j in range(len(pairs) + 1):
                    boundary = j > 0 and pairs[j - 1][1] == NPAIRS - 1
                    if boundary:
                        emit_out(j - 1)
                        emit_fin_mm(pairs[j - 1][0])
                        if j == len(pairs):
                            emit_fin_copy(pairs[j - 1][0])
                    if j < len(pairs):
                        mt, p = pairs[j]
                        # elementwise chain queued BEFORE prologue batches so
                        # early exps aren't stuck behind big conv copies
                        g_q[j] = emit_e_elem(j)
                        if boundary:
                            # fo-copy after the new exp: frees y banks well
                            # before out(j) needs them, without delaying exp
                            emit_fin_copy(pairs[j - 1][0])
                        if mt == 0:
                            if p in (6, 10):
                                emit_x_batch(p // 4 + 1)
                                emit_qk_batch(p // 4 + 1)
                            if p % 2 == 0:
                                emit_ut_batch(p // 2)
                    if j > 0 and not boundary:
                        emit_out(j - 1)

            if reps == 1:
                body()
            else:
                with tc.For_i(0, reps, 1):
                    body()
    nc.compile()
    return nc


_NC_CACHE = {}


def _get_nc(reps=1, variant=None):
    key = (reps, variant or VARIANT)
    if key not in _NC_CACHE:
        _NC_CACHE[key] = build_nc(reps, variant)
    return _NC_CACHE[key]


def _prep_in_maps(inputs):
    x = np.ascontiguousarray(np.asarray(inputs["x"], dtype=np.float32))
    wq = np.asarray(inputs["query_weight"], np.float32)[:, :, 0, 0]
    bq = np.asarray(inputs["query_bias"], np.float32)
    wk = np.asarray(inputs["key_weight"], np.float32)[:, :, 0, 0]
    bk = np.asarray(inputs["key_bias"], np.float32)
    wv = np.asarray(inputs["value_weight"], np.float32)[:, :, 0, 0]
    bv = np.asarray(inputs["value_bias"], np.float32)
    wg = np.asarray(inputs["gamma_weight"], np.float32)[:, :, 0, 0]
    bg = np.asarray(inputs["gamma_bias"], np.float32)

    wgv = wg @ wv                       # fused value+gamma conv weights
    bvg = wg @ bv
    # fp8 path keeps ut unscaled (fp8 dynamic range); 1/N applied in epilogue
    usc = 1.0 if FP8 else 1.0 / N
    # bge rank-1 bias matmul operand: scaled so that epilogue-scale * it = bge
    bsc = float(N) if FP8 else 1.0
    shared = {
        "wqT": np.ascontiguousarray(wq.T).reshape(2, 128, CQ),
        "wkT": np.ascontiguousarray(wk.T).reshape(2, 128, CQ),
        "bq": np.ascontiguousarray(bq.reshape(CQ, 1)),
        "bk": np.ascontiguousarray(bk.reshape(CQ, 1)),
        "wgvT": np.ascontiguousarray(wgv.T * usc).reshape(2, 128, C),
        "bvg": np.ascontiguousarray((bvg * usc).reshape(1, C)),
        "onesN": np.ones((1, N), ml_dtypes.bfloat16),
        "ones128": np.ones((1, 128), np.float32),
    }
    maps = []
    for b in range(B):
        xb = x[b].reshape(C, N)
        xsum = xb.sum(1)
        bge = bg - (wgv @ xsum) / N - bvg
        maps.append(dict(
            shared,
            x=xb.reshape(2, 128, N),
            bgeN=np.ascontiguousarray(
                (bge * bsc).reshape(1, C)).astype(ml_dtypes.bfloat16),
        ))
    return maps


def _run(inputs, trace=False, reps=1, variant=None):
    if variant is not None:
        global VARIANT
        VARIANT = variant
    nc = _get_nc(reps, variant)
    in_maps = _prep_in_maps(inputs)
    res = run_bass_kernel_spmd(nc, in_maps, core_ids=list(range(NCORES)),
                               trace=trace)
    out = np.stack([r["out"].transpose(1, 0, 2).reshape(C, HH, WW)
                    for r in res.results], axis=0)
    return out, res


def kernel(**inputs):
    out, _ = _run(inputs, trace=False)
    return out

